# revision 47
# baseline (speedup 1.0000x reference)
"""Trainium2 Bass kernel for nn_FragAttention (segment_reduce).

Reference computation (S=128, B=512, D=512, G=S-1=127):
    xb     = transpose(x, (1,0,2))            # (B, S, D)
    xm     = xb * (~src_mask)[:, :, None]     # zero padded tokens
    left   [b,g,d] = sum_{s<=g} xm[b,s,d]     # masked prefix sums
    right  [b,g,d] = sum_{s>g}  xm[b,s,d]
    out    = concat([left, right], axis=2)    # (B, G, 2D)

Strategy: pure data parallel over B across 8 cores (64 batches each).
The pad mask is folded into x on the host (exact: multiply by 0/1), and
x is cast to bf16 on the host — halves input HBM traffic; the 0/1
triangular weights are exact in bf16 so only x's mantissa truncation
matters (~2e-3 rel err, gate is 2e-2). Per batch the prefix/suffix sums
are computed on the TensorEngine as two matmuls against constant 0/1
triangular matrices (contraction over S=128 on partitions, f32 PSUM
accumulate), then PSUM->SBUF copies (DVE for even batches, ACT for odd)
cast the result to bf16. The host upcasts the returned bf16 block.

DMA (the roofline resource: ~25 MB/core at ~358 GB/s HBM): a transfer's
per-partition descriptors are sprayed across all 16 SDMA engines ONLY
when the SBUF-side AP covers all 128 partitions (8 partitions per
engine); any 127-partition AP (G=127!) falls off the swizzle path and
the whole DMA binds to ONE engine (~23 GB/s - this was the previous
335us bottleneck, on both the SWDGE and HWDGE paths). So the output
DRAM tensor is padded to 128 g-rows (row 127 is the valid g=127 full
sum, dropped by the host) and every DMA moves 128 partitions: writes
via SWDGE (gpsimd) hit ~340 GB/s aggregate, reads via the scalar(ACT)
HWDGE ring ~300+ GB/s, together peaking at the ~430 GB/s fabric limit.
All 4 input chunks are issued eagerly up front so read traffic is done
before writes ramp up. Output is written g-major (out[g, b, 2D]) so one
partition row is a 128 KB contiguous DRAM run: OUT_CHUNK=4 batches ->
128 descriptors of 8 KB per DMA. With DMA solved, the critical path is
read0-landing (~13us) + the tensor-paced compute stream (~2.6us per
4-batch chunk; at full clock the PE streams matmul columns at ~100%
efficiency, so the stream floor is clock-bound). Measured ~72-73us/core
in clean windows vs a ~70us roofline; slower runs are NC utilization
throttling (DVFS), visible in the NTFF throttle_* summary fields.
"""

import numpy as np
import ml_dtypes

import concourse.bass as bass
import concourse.mybir as mybir
from concourse import bacc
from concourse.tile import TileContext
from concourse.bass_utils import run_bass_kernel_spmd

S, B, D = 128, 512, 512
G = S - 1
N_CORES = 8
BL = B // N_CORES  # 64 batches per core

IN_CHUNK = 16  # batches per input DMA  (16 KB per-partition descriptors)
OUT_CHUNK = 4  # batches per output DMA (8 KB per-partition descriptors)

_NC_CACHE = None


def _build_bass() -> bass.Bass:
    nc = bacc.Bacc()
    f32 = mybir.dt.float32
    bf16 = mybir.dt.bfloat16

    x_in = nc.declare_dram_parameter("x", [S, BL, D], bf16, isOutput=False)
    # tri[:, 0:128] = upper (incl diag)  tri[s,g] = 1 if s <= g  -> prefix sums
    # tri[:, 128:256] = strictly lower   tri[s,g] = 1 if s >  g  -> suffix sums
    t_in = nc.declare_dram_parameter("tri", [S, 2 * S], bf16, isOutput=False)
    # g-major per-core output: partition row g maps to a contiguous DRAM run,
    # host transposes (G, BL, 2D) -> (BL, G, 2D) while gathering.
    # Padded to S=128 rows: a 128-partition SBUF side lets the DGE spray the
    # transfer's descriptors across all 16 SDMA engines (8 partitions per
    # engine); a 127-partition AP falls off the swizzle path and the whole
    # DMA binds to ONE engine (~23 GB/s). Row 127 is garbage; host drops it.
    out = nc.declare_dram_parameter("out", [S, BL, 2 * D], bf16, isOutput=True)

    with TileContext(nc) as tc:
        with (
            tc.tile_pool(name="const", bufs=1) as cpool,
            tc.tile_pool(name="xin", bufs=4) as xpool,
            tc.tile_pool(name="outs", bufs=8) as opool,
            tc.tile_pool(name="psum", bufs=4, space="PSUM") as ppool,
        ):
            tri = cpool.tile([S, 2 * S], bf16)
            nc.sync.dma_start(out=tri[:], in_=t_in[:])
            ut = tri[:, 0:S]        # (128, 128) stationary, left sums
            lt = tri[:, S : 2 * S]  # (128, 128) stationary, right sums

            def per_pair(xt, ot, j, k, use_dve):
                """2 batches (j, j+1) of xt -> slots (k, k+1) of ot.

                One 2-bank PSUM tile per batch (a matmul's output cannot
                exceed one 512-f32 PSUM bank per partition, so batches
                cannot share a wider matmul); bufs=4 keeps the tensor
                engine 4 batches ahead of the copies, removing
                chunk-boundary stalls. The copy engine alternates per ot
                tile (DVE for even chunks, ACT for odd): Tile tracks ot
                writes at tile granularity, so two engines writing disjoint
                slices of the same tile serialize on a false dependency -
                one tile, one engine keeps both copy engines fully parallel
                across chunks.
                """
                for c in range(2):
                    ps = ppool.tile([S, 2, D], f32)  # 2 adjacent banks
                    for h, tri_ in enumerate([ut, lt]):
                        nc.tensor.matmul(out=ps[:, h, :], lhsT=tri_,
                                         rhs=xt[:, j + c, :],
                                         start=True, stop=True)
                    dst = ot[:, k + c, :].rearrange("g (h d) -> g h d", h=2)
                    if use_dve:
                        nc.vector.tensor_copy(out=dst, in_=ps[:, :, :])
                    else:
                        nc.scalar.activation(
                            out=dst, in_=ps[:, :, :],
                            func=mybir.ActivationFunctionType.Copy,
                        )

            # issue all input loads eagerly (ACT HWDGE ring, sprays all 16
            # engines) so read traffic is done (~31us) before writes ramp
            # up. The critical path is read0-landing + tensor stream, so
            # read0 is slightly smaller (lands ~13us instead of ~15.3us);
            # later reads are sized so every chunk's data still lands >=2.7us
            # before the tensor stream reaches it (4/8-batch first reads and
            # 3/5-read plans break that margin: the read drain slows once
            # writes start, starving mid-stream compute).
            READS = [(0, 12), (12, 16), (28, 16), (44, 20)]
            xts = {}  # batch index of chunk start -> (tile, base batch)
            for r0, rn in READS:
                xt = xpool.tile([S, rn, D], bf16)
                nc.scalar.dma_start(out=xt[:], in_=x_in[:, r0 : r0 + rn, :])
                for b in range(r0, r0 + rn, 2):
                    xts[b] = (xt, r0)

            # Uniform 4-batch chunks (8 KB per-partition descriptors).
            # 2-batch chunks at the ends tested neutral: the earlier first
            # write is absorbed by the extra chunk boundaries, and the
            # drain is bounded by per-DMA latency (~6us sprayed + 2us
            # completion receipt), not by the final DMA's size.
            CHUNKS = [OUT_CHUNK] * (BL // OUT_CHUNK)
            o0 = 0
            for ci, csize in enumerate(CHUNKS):
                xt, xbase = xts[o0]
                ot = opool.tile([S, csize, 2 * D], bf16)
                for j in range(0, csize, 2):
                    per_pair(xt, ot, o0 - xbase + j, j, ci % 2 == 0)
                nc.gpsimd.dma_start(
                    out=out[:, o0 : o0 + csize, :], in_=ot[:, :, :],
                )
                o0 += csize
    nc.finalize()  # runs the Bacc pass pipeline (reg alloc, wait splitting)
    return nc


def _get_nc() -> bass.Bass:
    global _NC_CACHE
    if _NC_CACHE is None:
        _NC_CACHE = _build_bass()
    return _NC_CACHE


def _make_in_maps(x: np.ndarray, src_mask: np.ndarray) -> list[dict]:
    x = np.asarray(x, dtype=np.float32)
    src_mask = np.asarray(src_mask)
    assert x.shape == (S, B, D), x.shape
    assert src_mask.shape == (B, S), src_mask.shape

    valid = (~src_mask.astype(bool)).astype(np.float32).T  # (S, B)
    xm = (x * valid[:, :, None]).astype(ml_dtypes.bfloat16)
    tri = np.concatenate(
        [
            np.triu(np.ones((S, S), np.float32)),       # s <= g
            np.tril(np.ones((S, S), np.float32), -1),   # s >  g
        ],
        axis=1,
    ).astype(ml_dtypes.bfloat16)

    in_maps = []
    for i in range(N_CORES):
        sl = slice(i * BL, (i + 1) * BL)
        in_maps.append(
            {
                "x": np.ascontiguousarray(xm[:, sl, :]),
                "tri": tri,
            }
        )
    return in_maps


def _assemble(results: list[dict]) -> np.ndarray:
    full = np.empty((B, G, 2 * D), dtype=np.float32)
    for i in range(N_CORES):
        full[i * BL : (i + 1) * BL] = (
            results[i]["out"][:G].transpose(1, 0, 2).astype(np.float32)
        )
    return full


def kernel(x: np.ndarray, src_mask: np.ndarray) -> np.ndarray:
    in_maps = _make_in_maps(x, src_mask)
    res = run_bass_kernel_spmd(_get_nc(), in_maps, core_ids=list(range(N_CORES)))
    return _assemble(res.results)
